# revision 14
# baseline (speedup 1.0000x reference)
"""DecoderRNN Trainium2 kernel: 63-step LSTM + Luong attention + vocab projection.

Strategy (8 NeuronCores, SPMD), fp16 datapath (c-state/PSUM/softmax in f32):
  - Recurrence TP=8 over gate dims: each core owns 128 hidden dims x 4 gates
    (quarter order i,f,o,g so one sigmoid ACT covers i|f|o). Gates accumulate in
    ONE psum tile [P, 4B]; precomputed XgT enters via an identity-matmul.
  - Per-step AllGather payload [P, 32+n]: fp16 h-slice + up to 64 freshly
    computed dect cols piggybacked -> no separate in-loop dect collectives.
    The critical hall readback is split in two k-halves so the gates matmul
    starts on k=0..3 while k=4..7 is still landing.
  - Attention + W_w decoder in t-blocks as PE filler inside AllGather gaps;
    W_w output sharded by hidden chunk per core. Scores-layout enc tiles are
    streamed per step (not resident) to make room for W_out.
  - Vocab projection V-sharded (4000 cols/core) with W_out RESIDENT in SBUF
    (loaded once); logits written as fp16, b_out added host-side.
  - Tail: two post AllGathers (block-3 dect, then block-4 after its
    attention) with vocab interleaved.
"""

import numpy as np
import ml_dtypes
from contextlib import ExitStack

import concourse.bass as bass
import concourse.bacc as bacc
import concourse.tile as tile
import concourse.mybir as mybir
from concourse import masks
from concourse.bass_utils import run_bass_kernel_spmd

F32 = mybir.dt.float32
F16 = mybir.dt.float16
AF = mybir.ActivationFunctionType
ALU = mybir.AluOpType

B, T, S = 32, 63, 64
V, E, H = 32000, 512, 1024
P = 128
NCORES = 8
R = T * B                      # 2016 rows, r = t*B + b
VL = V // NCORES               # 4000
KH = H // P                    # 8
KE = E // P                    # 4
CH = 4                         # owned gate chunks (i,f,o,g quarters)
NT = 500                       # vocab n-tile width
VN = VL // NT                  # 8
Q_ORDER = [0, 1, 3, 2]         # quarter -> pytorch gate index (i,f,o,g)

# attention blocks (start, end); block 4 handled post-loop
BLOCKS = [(0, 16), (16, 32), (32, 44), (44, 56), (56, 63)]
SPREADS = [4, 4, 4, 8, 32]
TGROUPS = [(4 * i, min(4 * i + 4, T)) for i in range(16)]

# ---------------- static schedule ----------------


def build_schedule():
    scores = {}    # step -> list of (blk_idx, j)
    ctxs = {}      # step -> list of (blk_idx, jpair)
    dec = {}       # step -> blk_idx
    blk_dec = {}
    post_blocks = []
    for bi, (a, bnd) in enumerate(BLOCKS):
        start = bnd + 1
        spread = SPREADS[bi]
        steps_needed = (32 + spread - 1) // spread
        dstep = start + steps_needed + 1
        if dstep >= T:
            post_blocks.append(bi)
            continue
        for j in range(32):
            st = start + j // spread
            scores.setdefault(st, []).append((bi, j))
            if j % 4 == 3:
                ctxs.setdefault(st + 1, []).append((bi, j // 4))
        dec[dstep] = bi
        blk_dec[bi] = dstep

    # piggyback schedule: dect cols of block bi available from blk_dec[bi]+1
    avail = []
    for bi, (a, bnd) in enumerate(BLOCKS):
        if bi in blk_dec:
            avail.append((blk_dec[bi] + 1, a * B, (bnd - a) * B))
    pb = {}          # step -> (col0, n)
    col_ready = {}   # col -> AG step it rides
    pend = []
    ai = 0
    for t in range(T):
        while ai < len(avail) and avail[ai][0] <= t:
            pend.append([avail[ai][1], avail[ai][2]])
            ai += 1
        if pend:
            c0, n = pend[0][0], min(64, pend[0][1])
            pb[t] = (c0, n)
            for c in range(c0, c0 + n):
                col_ready[c] = t
            pend[0][0] += n
            pend[0][1] -= n
            if pend[0][1] == 0:
                pend.pop(0)
    post_c0 = R
    if pend:
        post_c0 = min(p[0] for p in pend)
    if ai < len(avail):
        post_c0 = min([post_c0] + [a[1] for a in avail[ai:]])
    for bi in post_blocks:
        a, bnd = BLOCKS[bi]
        post_c0 = min(post_c0, a * B)
    # group readiness
    vocab = {}
    post_groups = []
    items = []
    for g, (ta, tb) in enumerate(TGROUPS):
        cols = range(ta * B, tb * B)
        if all(c in col_ready for c in cols) and ta * B < post_c0:
            rdy = max(col_ready[c] for c in cols) + 2
            if rdy < T:
                items.append((rdy, g))
            else:
                post_groups.append(g)
        else:
            post_groups.append(g)
    items.sort()
    qi = 0
    queue = []
    for t in range(T):
        while qi < len(items) and items[qi][0] <= t:
            g = items[qi][1]
            queue.extend((g, n) for n in range(VN))
            qi += 1
        cnt = 0
        while queue and cnt < 3:
            vocab.setdefault(t, []).append(queue.pop(0))
            cnt += 1
    tail_vocab = list(queue) + [(items[i][1], n) for i in range(qi, len(items))
                                for n in range(VN)]
    return (scores, ctxs, dec, pb, post_c0, vocab, tail_vocab, post_groups,
            post_blocks)


(ATTN_SCHED, CTX_SCHED, DEC_SCHED, PB_SCHED, POST_C0, VOCAB_SCHED,
 TAIL_VOCAB, POST_GROUPS, POST_BLOCKS) = build_schedule()
POST1_C1 = BLOCKS[POST_BLOCKS[-1]][0] * B   # end of first post AG range
STAGEA_STEPS = {2: 1, 8: 2, 12: 3}   # step -> stage-A window (window 0 pre-loop)
AW = [(0, 512), (512, 1024), (1024, 1536), (1536, 2016)]

# enc prefetch for scores: step -> (bi, j0, nj); js per step are consecutive
ENC_PREF = {}
for _t, _lst in ATTN_SCHED.items():
    _bi = _lst[0][0]
    _js = [j for _, j in _lst]
    ENC_PREF[_t] = (_bi, min(_js), len(_js))


def build_graph():
    nc = bacc.Bacc("TRN2", target_bir_lowering=False, debug=False,
                   num_devices=NCORES)

    def inp(name, shape, dtype):
        return nc.dram_tensor(name, list(shape), dtype, kind="ExternalInput").ap()

    x_embT = inp("x_embT", [E, R], F16)
    wih_s = inp("wih_s", [E, CH * P], F16)
    whh_s = inp("whh_s", [H, CH * P], F16)
    bias_s = inp("bias_s", [P, CH], F32)
    h0T = inp("h0T", [H, B], F16)
    c0T_s = inp("c0T_s", [P, B], F32)
    encT_r = inp("encT_r", [P, B * KH * S], F16)   # [p, b, k, s]
    enc_r = inp("enc_r", [B, S, H], F16)
    ww_s = inp("ww_s", [2 * H, P], F16)            # W_w.T cols for own mo chunk
    bw_s = inp("bw_s", [P, 1], F32)
    wout_s = inp("wout_s", [H, VL], F16)
    out_s = nc.dram_tensor("out_s", [B, T, VL], F16, kind="ExternalOutput").ap()
    encT_4d = encT_r.rearrange("p (b k s) -> p b k s", b=B, k=KH)

    with tile.TileContext(nc) as tc, ExitStack() as ctx:
        pool1 = ctx.enter_context(tc.tile_pool(name="pool1", bufs=1))
        stream = ctx.enter_context(tc.tile_pool(name="stream", bufs=3))
        work = ctx.enter_context(tc.tile_pool(name="work", bufs=2))
        state = ctx.enter_context(tc.tile_pool(name="state", bufs=2))
        psp = ctx.enter_context(tc.tile_pool(name="psp", bufs=1, space="PSUM"))
        dram = ctx.enter_context(tc.tile_pool(name="dram", bufs=1, space="DRAM"))

        # ---------------- resident tiles ----------------
        hall = pool1.tile([P, KH, R], F16, name="hall")
        hall4 = hall.rearrange("p k (t b) -> p k t b", b=B)
        dectT = pool1.tile([P, KH, R], F16, name="dectT")
        dect_own = pool1.tile([P, R], F16, name="dect_own")
        whh = pool1.tile([P, KH, CH * P], F16, name="whh")
        nc.gpsimd.dma_start(whh[:], whh_s.rearrange("(k p) c -> p k c", p=P))
        wih = pool1.tile([P, KE, CH * P], F16, name="wih")
        nc.scalar.dma_start(wih[:], wih_s.rearrange("(k p) c -> p k c", p=P))
        bias_t = pool1.tile([P, CH], F32, name="bias_t")
        nc.sync.dma_start(bias_t[:], bias_s[:])
        wout_sb = pool1.tile([P, KH, VL], F16, name="wout_sb")
        wout_4d = wout_s.rearrange("(k p) v -> p k v", p=P)
        ww_sb = pool1.tile([P, 2 * KH, P], F16, name="ww_sb")
        nc.gpsimd.dma_start(ww_sb[:], ww_s.rearrange("(j p) m -> p j m", p=P))
        bw_t = pool1.tile([P, 1], F32, name="bw_t")
        nc.sync.dma_start(bw_t[:], bw_s[:])
        h0_t = pool1.tile([P, KH, B], F16, name="h0_t")
        nc.sync.dma_start(h0_t[:], h0T.rearrange("(k p) b -> p k b", p=P))
        ident = pool1.tile([P, P], F16, name="ident")
        masks.make_identity(nc, ident[:])
        c0_sb = pool1.tile([P, B], F32, name="c0_sb")
        nc.sync.dma_start(c0_sb[:], c0T_s[:])

        xg_dram = dram.tile([CH, P, R], F16, name="xg_dram")
        ccw = {t: B + (PB_SCHED[t][1] if t in PB_SCHED else 0)
               for t in range(T)}
        cc_in = [dram.tile([P, ccw[i]], F16, name=f"cc_in{i}")
                 for i in range(T)]
        cc_out = [dram.tile([NCORES * P, ccw[i]], F16, name=f"cc_out{i}",
                            addr_space="Shared") for i in range(T)]
        n1 = POST1_C1 - POST_C0
        n2 = R - POST1_C1
        pag1_in = dram.tile([P, n1], F16, name="pag1_in")
        pag1_out = dram.tile([NCORES * P, n1], F16, name="pag1_out",
                             addr_space="Shared")
        pag2_in = dram.tile([P, n2], F16, name="pag2_in")
        pag2_out = dram.tile([NCORES * P, n2], F16, name="pag2_out",
                             addr_space="Shared")

        # ---------------- helpers ----------------
        def stage_a(w):
            a, bnd = AW[w]
            nw = bnd - a
            xt = stream.tile([P, KE, 512], F16, name="xa", tag="xa", bufs=2)
            nc.gpsimd.dma_start(xt[:, :, :nw],
                                x_embT.rearrange("(k p) r -> p k r", p=P)[:, :, a:bnd])
            for c in range(CH):
                ps = psp.tile([P, 512], F32, name="ps_a", tag="mm")
                for k in range(KE):
                    nc.tensor.matmul(ps[:, :nw], lhsT=wih[:, k, c * P:(c + 1) * P],
                                     rhs=xt[:, k, :nw],
                                     start=(k == 0), stop=(k == KE - 1))
                xga = work.tile([P, 512], F16, name="xga", tag="xga", bufs=2)
                nc.scalar.activation(xga[:, :nw], ps[:, :nw], AF.Identity,
                                     bias=bias_t[:, c:c + 1])
                nc.gpsimd.dma_start(xg_dram[c, :, a:bnd], xga[:, :nw])

        def xg_prefetch(t):
            xg = stream.tile([P, CH, B], F16, name="xg", tag="xg", bufs=4)
            nc.gpsimd.dma_start(
                xg[:], xg_dram[:, :, t * B:(t + 1) * B].rearrange("c p b -> p c b"))
            return xg

        ec_tiles = {}
        enc_tiles = {}
        pn4_tiles = {}
        sc_tiles = {}
        at_tiles = {}
        softmax_fin = []

        def enc_prefetch(t):
            bi, j0, nj = ENC_PREF[t]
            if (bi, j0) in enc_tiles:
                return
            et = stream.tile([P, 8, KH, S], F16, name="encs", tag="encs",
                             bufs=2)
            nc.gpsimd.dma_start(et[:, :nj, :, :], encT_4d[:, j0:j0 + nj])
            enc_tiles[(bi, j0)] = (et, j0)

        def attn_scores(bi, j, et, ej0):
            blk_a, blk_b = BLOCKS[bi]
            w = blk_b - blk_a
            if j % 2 == 0 and (bi, j // 2) not in ec_tiles:
                b0 = j
                ec = stream.tile([2 * S, H], F16, name="ec", tag="ec", bufs=4)
                nc.gpsimd.dma_start(ec[0:S, :], enc_r[b0, :, :])
                nc.gpsimd.dma_start(ec[S:2 * S, :], enc_r[b0 + 1, :, :])
                ec_tiles[(bi, j // 2)] = ec
            q = j // 4
            if (bi, q) not in sc_tiles:
                sc_tiles[(bi, q)] = psp.tile([P, S], F32, name="ps_sc",
                                             tag="sc", bufs=2)
            ps4 = sc_tiles[(bi, q)]
            jj = j % 4
            for k in range(KH):
                nc.tensor.matmul(ps4[32 * jj:32 * jj + w, :],
                                 lhsT=hall4[:, k, blk_a:blk_b, j],
                                 rhs=et[:, j - ej0, k, :],
                                 start=(k == 0), stop=(k == KH - 1),
                                 tile_position=(0, 32 * jj),
                                 skip_group_check=True)
            if jj == 3:
                ps4 = sc_tiles.pop((bi, q))
                probs = work.tile([P, S], F32, name="probs", tag="probs",
                                  bufs=3)
                ssum = work.tile([P, 1], F32, name="ssum", tag="ssum", bufs=3)
                nc.scalar.activation(probs[:], ps4[:], AF.Exp,
                                     accum_out=ssum[:])
                softmax_fin.append((bi, q, probs, ssum))

        def flush_softmax():
            recs = []
            for (bi, q, probs, ssum) in softmax_fin:
                rec = work.tile([P, 1], F32, name="rec", tag="rec", bufs=4)
                nc.vector.reciprocal(rec[:], ssum[:])
                recs.append(rec)
            for (bi, q, probs, ssum), rec in zip(softmax_fin, recs):
                pn4 = work.tile([P, S], F16, name="pn4", tag="pn4", bufs=4)
                nc.scalar.mul(pn4[:], probs[:], rec[:])
                pn4_tiles[(bi, q)] = pn4
            softmax_fin.clear()

        def ctx_prep(bi, q):
            pn4 = pn4_tiles.pop((bi, q))
            ps_at = psp.tile([P, P], F16, name="ps_at", tag="at")
            nc.tensor.transpose(ps_at[0:S, :], pn4[:, :], ident[:, :])
            attnT = work.tile([P, P], F16, name="attnT", tag="attnT", bufs=3)
            nc.vector.tensor_copy(attnT[0:S, :], ps_at[0:S, :])
            nc.vector.tensor_copy(attnT[S:2 * S, :], ps_at[0:S, :])
            at_tiles[(bi, q)] = attnT

        def ctx_main(bi, q):
            blk_a, blk_b = BLOCKS[bi]
            w = blk_b - blk_a
            attnT = at_tiles.pop((bi, q))
            for jj in range(4):
                j = q * 4 + jj
                half = j % 2
                ec = ec_tiles[(bi, j // 2)]
                ps_cx = psp.tile([P, KH, 16], F32, name="ps_cx", tag="cx")
                for k in range(KH):
                    nc.tensor.matmul(ps_cx[:, k, :w],
                                     lhsT=ec[half * S:(half + 1) * S,
                                             k * P:(k + 1) * P],
                                     rhs=attnT[half * S:(half + 1) * S,
                                               32 * jj:32 * jj + w],
                                     start=True, stop=True)
                cxr = ctx_blk.rearrange("p k (t b) -> p k t b", b=B)
                nc.vector.tensor_copy(cxr[:, :, :w, j], ps_cx[:, :, :w])
                if half == 1:
                    ec_tiles.pop((bi, j // 2))

        def dec_blk(bi):
            blk_a, blk_b = BLOCKS[bi]
            w = blk_b - blk_a
            ps_d = psp.tile([P, 512], F32, name="ps_d", tag="mm")
            for j in range(2 * KH):
                rhs = (hall[:, j, blk_a * B:blk_b * B] if j < KH
                       else ctx_blk[:, j - KH, :w * B])
                nc.tensor.matmul(ps_d[:, :w * B], lhsT=ww_sb[:, j, :], rhs=rhs,
                                 start=(j == 0), stop=(j == 2 * KH - 1))
            nc.scalar.activation(dect_own[:, blk_a * B:blk_b * B], ps_d[:, :w * B],
                                 AF.Tanh, bias=bw_t[:, 0:1])

        vcnt = [0]

        def vocab_chunk(g, n):
            ta, tb = TGROUPS[g]
            mw = (tb - ta) * B
            ps_v = psp.tile([P, NT], F32, name="ps_v", tag="pv", bufs=2)
            for k in range(KH):
                nc.tensor.matmul(ps_v[:mw, :], lhsT=dectT[:, k, ta * B:tb * B],
                                 rhs=wout_sb[:, k, n * NT:(n + 1) * NT],
                                 start=(k == 0), stop=(k == KH - 1))
            o_sb = work.tile([P, NT], F16, name="o_sb", tag="o_sb", bufs=3)
            if vcnt[0] % 2 == 0:
                nc.scalar.copy(o_sb[:mw, :], ps_v[:mw, :])
            else:
                nc.vector.tensor_copy(o_sb[:mw, :], ps_v[:mw, :])
            vcnt[0] += 1
            nc.gpsimd.dma_start(
                out_s[:, ta:tb, n * NT:(n + 1) * NT].transpose([1, 0, 2]),
                o_sb[:mw, :])

        # ---------------- pre-loop ----------------
        ctx_blk = pool1.tile([P, KH, 16 * B], F16, name="cxb")
        stage_a(0)
        xg_q = {0: xg_prefetch(0), 1: xg_prefetch(1)}

        # ---------------- main loop ----------------
        c_prev = c0_sb
        for t in range(T):
            for (bi, jp) in CTX_SCHED.get(t, []):
                ctx_prep(bi, jp)
            # gates: psum [P, 4B]; identity-matmul folds Xg in
            psg = psp.tile([P, CH * B], F32, name="psg", tag="psg", bufs=1)
            xg = xg_q.pop(t)
            nc.tensor.matmul(psg[:], lhsT=ident[:],
                             rhs=xg[:].rearrange("p c b -> p (c b)"),
                             start=True, stop=False, skip_group_check=True)
            for k in range(KH):
                for qq in range(CH):
                    rhs = (h0_t[:, k, :] if t == 0 else
                           hall4[:, k, t - 1, :])
                    nc.tensor.matmul(psg[:, qq * B:(qq + 1) * B],
                                     lhsT=whh[:, k, qq * P:(qq + 1) * P],
                                     rhs=rhs, start=False,
                                     stop=(qq == CH - 1 and k == KH - 1),
                                     skip_group_check=True)
            sfo = work.tile([P, 3 * B], F32, name="sfo", tag="sfo")
            nc.scalar.activation(sfo[:], psg[:, 0:3 * B], AF.Sigmoid)
            tg = work.tile([P, B], F32, name="tg", tag="tg")
            nc.scalar.activation(tg[:], psg[:, 3 * B:4 * B], AF.Tanh)
            t1 = work.tile([P, B], F32, name="t1", tag="t1")
            nc.vector.tensor_mul(t1[:], sfo[:, B:2 * B], c_prev[:])
            t2 = work.tile([P, B], F32, name="t2", tag="t2")
            nc.vector.tensor_mul(t2[:], sfo[:, 0:B], tg[:])
            c_new = state.tile([P, B], F32, name="c_new", tag="c_new")
            nc.vector.tensor_add(c_new[:], t1[:], t2[:])
            c_prev = c_new
            tc_t = work.tile([P, B], F32, name="tc_t", tag="tc_t")
            nc.scalar.activation(tc_t[:], c_new[:], AF.Tanh)
            h16 = work.tile([P, B], F16, name="h16", tag="h16")
            nc.vector.tensor_mul(h16[:], sfo[:, 2 * B:3 * B], tc_t[:])
            nc.sync.dma_start(cc_in[t][:, 0:B], h16[:])
            if t in PB_SCHED:
                c0p, np_ = PB_SCHED[t]
                nc.gpsimd.dma_start(cc_in[t][:, B:B + np_],
                                    dect_own[:, c0p:c0p + np_])
            nc.gpsimd.collective_compute(
                "AllGather", ALU.bypass,
                replica_groups=[list(range(NCORES))],
                ins=[cc_in[t].opt()], outs=[cc_out[t].opt()])
            # split hall readback: k=0..3 first (gates consume k-major)
            nc.sync.dma_start(
                hall4[:, 0:4, t, :],
                cc_out[t][0:4 * P, 0:B].rearrange("(k p) b -> p k b", p=P))
            nc.sync.dma_start(
                hall4[:, 4:8, t, :],
                cc_out[t][4 * P:8 * P, 0:B].rearrange("(k p) b -> p k b", p=P))
            if t in PB_SCHED:
                c0p, np_ = PB_SCHED[t]
                nc.gpsimd.dma_start(
                    dectT[:, :, c0p:c0p + np_],
                    cc_out[t][:, B:B + np_].rearrange("(k p) c -> p k c", p=P))

            # ---- filler ----
            if t + 2 < T:
                xg_q[t + 2] = xg_prefetch(t + 2)
            if 1 <= t <= 8:
                n0 = (t - 1) * NT
                nc.scalar.dma_start(wout_sb[:, :, n0:n0 + NT],
                                    wout_4d[:, :, n0:n0 + NT])
            if t in STAGEA_STEPS:
                stage_a(STAGEA_STEPS[t])
            if t in ATTN_SCHED:
                enc_prefetch(t)
                bi, j0, nj = ENC_PREF[t]
                et, ej0 = enc_tiles.pop((bi, j0))
                for (bi2, j) in ATTN_SCHED[t]:
                    attn_scores(bi2, j, et, ej0)
            if t + 1 in ENC_PREF:
                enc_prefetch(t + 1)
            flush_softmax()
            for (bi, jp) in CTX_SCHED.get(t, []):
                ctx_main(bi, jp)
            if t in DEC_SCHED:
                dec_blk(DEC_SCHED[t])
            for (g, n) in VOCAB_SCHED.get(t, []):
                vocab_chunk(g, n)

        # ---------------- tail ----------------
        # post AG 1: block-3 dect (decoded at the last loop step)
        nc.scalar.dma_start(pag1_in[:], dect_own[:, POST_C0:POST1_C1])
        nc.gpsimd.collective_compute(
            "AllGather", ALU.bypass,
            replica_groups=[list(range(NCORES))],
            ins=[pag1_in.opt()], outs=[pag1_out.opt()])
        nc.sync.dma_start(
            dectT[:, :, POST_C0:POST1_C1],
            pag1_out[:].rearrange("(k p) c -> p k c", p=P))

        early_tail = [x for x in TAIL_VOCAB if x[0] not in POST_GROUPS]
        g1 = [g for g in POST_GROUPS if TGROUPS[g][0] * B < POST1_C1]
        g2 = [g for g in POST_GROUPS if TGROUPS[g][0] * B >= POST1_C1]
        tail1 = early_tail + [(g, n) for g in g1 for n in range(VN)]
        ei = 0
        for bi in POST_BLOCKS:
            for q in range(8):
                for jh in range(2):
                    j0 = q * 4 + jh * 2
                    et = stream.tile([P, 2, KH, S], F16, name="encp",
                                     tag="encp", bufs=3)
                    nc.gpsimd.dma_start(et[:], encT_4d[:, j0:j0 + 2])
                    enc_tiles[("post", j0)] = (et, j0)
                for jj in range(4):
                    j = q * 4 + jj
                    et, ej0 = enc_tiles[("post", (j // 2) * 2)]
                    attn_scores(bi, j, et, ej0)
                flush_softmax()
                ctx_prep(bi, q)
                ctx_main(bi, q)
                for _ in range(4):
                    if ei < len(tail1):
                        g, n = tail1[ei]
                        vocab_chunk(g, n)
                        ei += 1
            dec_blk(bi)
        # post AG 2: block-4 dect
        nc.scalar.dma_start(pag2_in[:], dect_own[:, POST1_C1:R])
        nc.gpsimd.collective_compute(
            "AllGather", ALU.bypass,
            replica_groups=[list(range(NCORES))],
            ins=[pag2_in.opt()], outs=[pag2_out.opt()])
        for (g, n) in tail1[ei:]:
            vocab_chunk(g, n)
        nc.sync.dma_start(
            dectT[:, :, POST1_C1:R],
            pag2_out[:].rearrange("(k p) c -> p k c", p=P))
        for g in g2:
            for n in range(VN):
                vocab_chunk(g, n)
    nc.compile()
    return nc


_CACHE = {}


def _get_graph():
    if "nc" not in _CACHE:
        _CACHE["nc"] = build_graph()
    return _CACHE["nc"]


def _prep(tgt_input, hidden_state, cell_state, encoder_outputs,
          embedding, W_ih, W_hh, b_ih, b_hh, W_w, b_w, W_out, b_out):
    f32 = np.float32
    f16 = np.float16
    idx = np.asarray(tgt_input)[:, :-1].astype(np.int64)
    emb = np.asarray(embedding, f32)[idx]                    # [B, T, E]
    x_embT = np.ascontiguousarray(
        emb.transpose(2, 1, 0).reshape(E, R)).astype(f16)
    w_ihT = np.asarray(W_ih, f32).T                          # [E, G]
    w_hhT = np.asarray(W_hh, f32).T                          # [H, G]
    bias = (np.asarray(b_ih, f32) + np.asarray(b_hh, f32))
    h0T = np.ascontiguousarray(np.asarray(hidden_state, f32)[0].T).astype(f16)
    c0T = np.ascontiguousarray(np.asarray(cell_state, f32)[0].T)   # [H, B]
    enc = np.asarray(encoder_outputs, f32)                   # [B, S, H]
    enc_r = enc.astype(f16)
    encT_r = np.ascontiguousarray(
        enc.transpose(2, 1, 0)                               # [H, S, B]
        .reshape(KH, P, S, B).transpose(1, 3, 0, 2)          # [P, B, KH, S]
        .reshape(P, B * KH * S)).astype(f16)
    w_wT = np.ascontiguousarray(np.asarray(W_w, f32).T)      # [2H, H]
    b_w_a = np.asarray(b_w, f32)
    w_outT = np.asarray(W_out, f32).T                        # [H, V]

    in_maps = []
    for m in range(NCORES):
        cols = np.concatenate([np.arange(Q_ORDER[q] * H + m * P,
                                         Q_ORDER[q] * H + m * P + P)
                               for q in range(4)])
        in_maps.append({
            "x_embT": x_embT,
            "wih_s": np.ascontiguousarray(w_ihT[:, cols]).astype(f16),
            "whh_s": np.ascontiguousarray(w_hhT[:, cols]).astype(f16),
            "bias_s": np.ascontiguousarray(bias[cols].reshape(CH, P).T),
            "h0T": h0T,
            "c0T_s": np.ascontiguousarray(c0T[m * P:(m + 1) * P, :]),
            "encT_r": encT_r,
            "enc_r": enc_r,
            "ww_s": np.ascontiguousarray(w_wT[:, m * P:(m + 1) * P]).astype(f16),
            "bw_s": np.ascontiguousarray(b_w_a[m * P:(m + 1) * P]).reshape(P, 1),
            "wout_s": np.ascontiguousarray(
                w_outT[:, m * VL:(m + 1) * VL]).astype(f16),
        })
    return in_maps


def kernel(**inputs) -> np.ndarray:
    nc = _get_graph()
    in_maps = _prep(**inputs)
    res = run_bass_kernel_spmd(nc, in_maps, list(range(NCORES)))
    outs = [res.results[m]["out_s"].astype(np.float32) for m in range(NCORES)]
    full = np.concatenate(outs, axis=2)
    full += np.asarray(inputs["b_out"], np.float32)[None, None, :]
    return full
